# revision 2
# baseline (speedup 1.0000x reference)
"""Trainium2 Bass kernel for nn_BertAttention_78554951843978 (v2).

Reference (B=2, S=2048, D=1024, H=16, hd=64, fp32):
    q = split_heads(hs @ Wq.T + bq); k,v likewise
    probs = softmax(q k^T / 8); ctx = probs @ v
    x = relu(merge_heads(ctx) + hs @ Wp.T)
    out = layernorm(x) * gamma + beta      (eps = 1e-12)

Sharding (8 cores): data-parallel over B (2 groups of 4), tensor-parallel
over heads within a group (4 heads / 256 dims per core).

Plan per core:
  - q/k/v projections + scores in fp8e4 with DoubleRow packing (weights
    prescaled x32 on host; scores psum = 1024 * q.k, exp scale folds it).
  - residual projection in fp16 (hs, Wp shipped fp16); v-bias folded into
    the residual cast (exact: ctx/denom + bv == avg(v + bv)).
  - exp split across engines: ACT (true Exp), DVE + GpSimd (Schraudolph
    int32 bit trick, bit pattern == f32), all consumed as f32r.
  - ctx = vA^T @ probs, ones-augmented V: even heads [v|1] (denominator in
    psum row 64), odd heads [1|v] (denominator row 0); odd-head ctx is
    staged with a partition-shifting DMA so both head halves divide and
    accumulate into x without any DVE partition shifts.
  - layernorm stats per 512-token chunk inside the attention loop; two
    AllReduces (tokens 0:1024, 1024:2048) so the first is fully hidden;
    rstd via rsqrt bit trick + 2 Newton steps (no ACT table swap).
"""

import numpy as np

import concourse.bass as bass
import concourse.tile as tile
from concourse import mybir
from concourse.bass_utils import run_bass_kernel_spmd

B, S, D, H = 2, 2048, 1024, 16
HD = 64
NCORES = 8
GROUPS = 4
DC = D // GROUPS          # 256 dims per core
EPS = 1e-12
WSCALE = 32.0             # host prescale of Wq/Wk/Wv for fp8 range

F32 = mybir.dt.float32
F32R = mybir.dt.float32r
F16 = mybir.dt.float16
F8 = mybir.dt.float8e4
I32 = mybir.dt.int32
AF = mybir.ActivationFunctionType
OP = mybir.AluOpType
PM = mybir.MatmulPerfMode

KT8 = 4          # fp8 DoubleRow contraction tiles (256 each)
KT16 = 8         # fp16 contraction tiles (128 each)
MT = 2           # 128-dim output tiles (head pairs)
NS = 4           # 512-token chunks
ST = 16          # 128-key tiles

# exp engine per key-tile: balance ACT / DVE / Pool
# exp engine per key-tile (256-query blocks): 7 ACT / 4 DVE / 5 Pool
EXP_ENG = ["act", "dve", "pool", "act", "pool", "act", "dve", "act",
           "pool", "act", "dve", "pool", "act", "dve", "act", "pool"]

# Schraudolph exp: i32 = x * (2^23/ln2)*scale + (127*2^23 - C)
EXPA = float((1 << 23) / np.log(2.0))
EXPB = float(127 * (1 << 23) - 366393)
SCL = 1.0 / (WSCALE * WSCALE * np.sqrt(HD))   # scores psum -> logits


def _split_waits(nc, keep=1):
    """Walrus in this container rejects >1 sem wait per (non-EVSEM)
    instruction; hoist extras onto single-wait NOPs on the same engine."""
    for bb in nc.main_func.blocks:
        insts = list(bb.instructions)
        out_list = []
        changed = False
        for inst in insts:
            si = inst.sync_info
            cap = 2 if isinstance(inst, mybir.InstEventSemaphore) else keep
            if si is not None and si.on_wait is not None and len(si.on_wait) > cap:
                waits = list(si.on_wait)
                for w in waits[cap:]:
                    out_list.append(mybir.InstNoOp(
                        name=nc.get_next_instruction_name(),
                        engine=inst.engine,
                        ins=[], outs=[],
                        sync_info=mybir.SyncInfo(on_wait=[w], on_update=[]),
                        bass_nofuse=True,
                    ))
                inst.sync_info = mybir.SyncInfo(
                    on_wait=waits[:cap], on_update=list(si.on_update or []))
                changed = True
            out_list.append(inst)
        if changed:
            bb.instructions = out_list


def build_bass(split_waits=True):
    nc = bass.Bass(num_devices=NCORES)

    # ---------------- DRAM I/O ----------------
    hs8_d = nc.dram_tensor("hs8", [KT8 * 128, 2 * S], F8, kind="ExternalInput")
    wq8_d = nc.dram_tensor("wq8", [KT8 * 128, 2 * DC], F8, kind="ExternalInput")
    wk8_d = nc.dram_tensor("wk8", [KT8 * 128, 2 * DC], F8, kind="ExternalInput")
    wv8_d = nc.dram_tensor("wv8", [KT8 * 128, 2 * DC], F8, kind="ExternalInput")
    hsT_d = nc.dram_tensor("hsT16", [D, S], F16, kind="ExternalInput")
    wpT_d = nc.dram_tensor("wp16", [D, DC], F16, kind="ExternalInput")
    bq_d = nc.dram_tensor("bq32", [DC], F32, kind="ExternalInput")   # *32
    bk_d = nc.dram_tensor("bk32", [DC], F32, kind="ExternalInput")   # *32
    bv_d = nc.dram_tensor("bv", [DC], F32, kind="ExternalInput")
    gm_d = nc.dram_tensor("gamma", [DC], F32, kind="ExternalInput")
    bt_d = nc.dram_tensor("beta", [DC], F32, kind="ExternalInput")
    out_d = nc.dram_tensor("outT", [DC, S], F32, kind="ExternalOutput")

    with tile.TileContext(nc) as tc:
        with (
            tc.tile_pool(name="persist", bufs=1) as persist,
            tc.tile_pool(name="dram", bufs=1, space="DRAM") as dram,
        ):
            # ------------- persistent SBUF -------------
            hs8 = persist.tile([128, KT8, 2, S], F8)            # 16 KB/part
            wq8 = persist.tile([128, KT8, 2, DC], F8)           # 2 KB
            wk8 = persist.tile([128, KT8, 2, DC], F8)
            wv8 = persist.tile([128, KT8, 2, DC], F8)
            hsT = persist.tile([128, KT16, S], F16)             # 32 KB
            wp = persist.tile([128, KT16, MT, 128], F16)        # 4 KB
            q8t = persist.tile([128, MT, S], F8)                # 4 KB (tmp)
            k8t = persist.tile([128, MT, S], F8)
            q8 = persist.tile([128, 2, S], F8)                  # head h @ parts 32h
            k8 = persist.tile([128, 2, S], F8)
            vA = persist.tile([128, ST, GROUPS, HD + 1], F32R)  # 16.25 KB
            x = persist.tile([128, MT, S], F32)                 # 16 KB
            dummy = persist.tile([128, 512], F32R)              # PE warmup
            cst = persist.tile([128, 76], F32)
            bq_s, bk_s = cst[:, 0:2], cst[:, 2:4]
            bv_s = cst[:, 4:6]
            gm_s, bt_s = cst[:, 6:8], cst[:, 8:10]
            onesr = cst[:, 10:11].bitcast(F32R)
            ones64 = cst[:, 12:76].bitcast(F32R)

            nc.vector.memset(dummy.bitcast(F32), 0.0)
            nc.vector.memset(cst[:, 10:76], 1.0)

            # input DMAs; chunked so the single DMA pipe interleaves
            hs8_t = hs8_d.rearrange("(t p) (j s) -> p t j s", p=128, j=2)
            wq8_t = wq8_d.rearrange("(t p) (j c) -> p t j c", p=128, j=2)
            wk8_t = wk8_d.rearrange("(t p) (j c) -> p t j c", p=128, j=2)
            wv8_t = wv8_d.rearrange("(t p) (j c) -> p t j c", p=128, j=2)
            hsT_t = hsT_d.rearrange("(t p) s -> p t s", p=128)
            wp_t = wpT_d.rearrange("(t p) (m f) -> p t m f", p=128, f=128)
            nc.sync.dma_start(out=wq8, in_=wq8_t)
            nc.sync.dma_start(out=wk8, in_=wk8_t)
            for t in range(KT8):
                nc.sync.dma_start(out=hs8[:, t], in_=hs8_t[:, t])
            nc.sync.dma_start(out=wv8, in_=wv8_t)
            nc.scalar.dma_start(out=bq_s, in_=bq_d.rearrange("(m p) -> p m", p=128))
            nc.scalar.dma_start(out=bk_s, in_=bk_d.rearrange("(m p) -> p m", p=128))
            nc.scalar.dma_start(out=bv_s, in_=bv_d.rearrange("(m p) -> p m", p=128))
            nc.scalar.dma_start(out=gm_s, in_=gm_d.rearrange("(m p) -> p m", p=128))
            nc.scalar.dma_start(out=bt_s, in_=bt_d.rearrange("(m p) -> p m", p=128))
            nc.sync.dma_start(out=wp, in_=wp_t)
            for t in range(0, KT16, 2):
                nc.sync.dma_start(out=hsT[:, t:t + 2, :], in_=hsT_t[:, t:t + 2, :])

            # DRAM scratch (separate tiles per collective: avoid false WAR)
            cc_in = [dram.tile([1, 2048], F32, name=f"cc_in{i}") for i in range(2)]
            cc_out = [dram.tile([1, 2048], F32, name=f"cc_out{i}") for i in range(2)]
            srow = [dram.tile([2, 1024], F32, name=f"srow{i}") for i in range(2)]

            with (
                tc.tile_pool(name="scps", bufs=2, space="PSUM") as scps,
                tc.tile_pool(name="ctxps", bufs=1, space="PSUM") as ctxps,
                tc.tile_pool(name="auxps", bufs=2, space="PSUM") as auxps,
                tc.tile_pool(name="ptp", bufs=4) as ptp,
                tc.tile_pool(name="stg", bufs=1) as stg,
                tc.tile_pool(name="small", bufs=2) as small,
                tc.tile_pool(name="x2p", bufs=2) as x2p,
                tc.tile_pool(name="rows", bufs=1) as rows,
                tc.tile_pool(name="abp", bufs=2) as abp,
                tc.tile_pool(name="ostg", bufs=2) as ostg,
            ):
                # ---------- PE warmup ----------
                warm = auxps.tile([128, 512], F32, name="warm", tag="aux")
                for i in range(26):
                    nc.tensor.matmul(out=warm, lhsT=dummy[:, 0:128],
                                     rhs=dummy, start=True, stop=True)

                # ---------- projection helpers ----------
                def proj_qk(w8, m, n, bias, out_sb):
                    """fp8 DoubleRow W-stationary projection block [128,512]."""
                    ps = auxps.tile([128, 512], F32, name="qkps", tag="aux")
                    for t in range(KT8):
                        nc.tensor.matmul(
                            out=ps, lhsT=w8[:, t, :, m * 128:(m + 1) * 128],
                            rhs=hs8[:, t, :, n * 512:(n + 1) * 512],
                            perf_mode=PM.DoubleRow,
                            start=(t == 0), stop=(t == KT8 - 1))
                    nc.scalar.activation(
                        out=out_sb[:, m, n * 512:(n + 1) * 512], in_=ps,
                        func=AF.Identity, bias=bias)

                def proj_v(tp):
                    """v for token-tiles 2tp, 2tp+1 (hs-stationary, DR)."""
                    ps = auxps.tile([128, 512], F32, name="vps", tag="aux")
                    for tt_ in range(2):
                        t0 = 2 * tp + tt_
                        for t in range(KT8):
                            nc.tensor.matmul(
                                out=ps[:, tt_ * 256:tt_ * 256 + 256],
                                lhsT=hs8[:, t, :, t0 * 128:(t0 + 1) * 128],
                                rhs=wv8[:, t],
                                perf_mode=PM.DoubleRow,
                                start=(t == 0), stop=(t == KT8 - 1))
                    # scatter into vA: all heads [v(64) | ones]
                    for tt_ in range(2):
                        t0 = 2 * tp + tt_
                        src = ps[:, tt_ * 256:tt_ * 256 + 256].rearrange(
                            "p (h d) -> p h d", d=HD)
                        if tt_ == 0:
                            nc.vector.tensor_scalar_mul(
                                out=vA[:, t0, :, 0:HD].bitcast(F32), in0=src,
                                scalar1=1.0 / WSCALE)
                        else:
                            nc.scalar.activation(
                                out=vA[:, t0, :, 0:HD].bitcast(F32), in_=src,
                                func=AF.Copy, scale=1.0 / WSCALE)
                    nc.gpsimd.memset(
                        vA[:, 2 * tp:2 * tp + 2, :, HD:HD + 1].bitcast(F32), 1.0)

                def proj_r(m, n):
                    """fp16 residual projection block into x (bias = bv)."""
                    ps = auxps.tile([128, 512], F32, name="rps", tag="aux")
                    for t in range(KT16):
                        nc.tensor.matmul(
                            out=ps, lhsT=wp[:, t, m, :],
                            rhs=hsT[:, t, n * 512:(n + 1) * 512],
                            start=(t == 0), stop=(t == KT16 - 1))
                    nc.scalar.activation(
                        out=x[:, m, n * 512:(n + 1) * 512], in_=ps,
                        func=AF.Identity, bias=bv_s[:, m:m + 1])

                def rsh_k(m):
                    nc.sync.dma_start(
                        out=k8[64 * m:64 * m + 64, :, :], in_=k8t[:, m, :])

                def rsh_q(m, n):
                    qs = slice(n * 512, (n + 1) * 512)
                    nc.sync.dma_start(
                        out=q8[64 * m:64 * m + 64, :, qs], in_=q8t[:, m, qs])

                # upfront: q(m0,n0), k(m0,*) + reshuffles
                proj_qk(wq8, 0, 0, bq_s[:, 0:1], q8t)
                for n in range(NS):
                    proj_qk(wk8, 0, n, bk_s[:, 0:1], k8t)
                rsh_k(0)
                rsh_q(0, 0)

                # per-block filler schedules (consumed one per key-tile)
                def F_v(tp):
                    return lambda: proj_v(tp)

                def F_qk(w8, m, n, bias, dst):
                    return lambda: proj_qk(w8, m, n, bias, dst)

                def F_r(m, n):
                    return lambda: proj_r(m, n)

                F000 = [F_v(t) for t in range(8)]
                F001 = [F_qk(wk8, 1, n, bk_s[:, 1:2], k8t) for n in range(NS)] \
                    + [F_qk(wq8, 1, 0, bq_s[:, 1:2], q8t),
                       lambda: rsh_k(1), lambda: rsh_q(1, 0)]
                F010 = [F_qk(wq8, 0, 1, bq_s[:, 0:1], q8t),
                        lambda: rsh_q(0, 1), F_r(0, 0)]
                F011 = [F_qk(wq8, 1, 1, bq_s[:, 1:2], q8t),
                        lambda: rsh_q(1, 1), F_r(1, 0)]
                F100 = [F_qk(wq8, 0, 2, bq_s[:, 0:1], q8t),
                        lambda: rsh_q(0, 2), F_r(0, 1)]
                F101 = [F_qk(wq8, 1, 2, bq_s[:, 1:2], q8t),
                        lambda: rsh_q(1, 2), F_r(1, 1)]
                F110 = [F_qk(wq8, 0, 3, bq_s[:, 0:1], q8t),
                        lambda: rsh_q(0, 3), F_r(0, 2)]
                F111 = [F_qk(wq8, 1, 3, bq_s[:, 1:2], q8t),
                        lambda: rsh_q(1, 3), F_r(1, 2)]
                F200 = [F_r(0, 3)]
                F201 = [F_r(1, 3)]

                # ---------- attention ----------
                pend = {}   # (qn, hp, sub) -> tmp tile awaiting residual

                def block(qn, hp, sub, fl):
                    qs = slice(qn * 512 + sub * 256, qn * 512 + sub * 256 + 256)
                    h0, h1 = 2 * hp, 2 * hp + 1
                    ctxA = ctxps.tile([128, 256], F32, name="ctxA")
                    ctxB = ctxps.tile([128, 256], F32, name="ctxB")

                    def ctx_mms(pt, ks):
                        nc.tensor.matmul(
                            out=ctxA[0:HD + 1, :],
                            lhsT=vA[:, ks, h0, :],
                            rhs=pt[:, 0:256],
                            start=(ks == 0), stop=(ks == ST - 1))
                        nc.tensor.matmul(
                            out=ctxB[0:HD + 1, :],
                            lhsT=vA[:, ks, h1, :],
                            rhs=pt[:, 256:512],
                            start=(ks == 0), stop=(ks == ST - 1))

                    lag = 4
                    pts = {}
                    for ks in range(ST):
                        if fl:
                            fl.pop(0)()
                        sc = scps.tile([128, 512], F32, name="sc", bufs=4)
                        kslc = slice(ks * 128, (ks + 1) * 128)
                        nc.tensor.matmul(
                            out=sc[:, 0:256],
                            lhsT=k8[32 * h0:32 * h0 + 32, :, kslc],
                            rhs=q8[32 * h0:32 * h0 + 32, :, qs],
                            perf_mode=PM.DoubleRow,
                            tile_position=(32 * h0, 0))
                        nc.tensor.matmul(
                            out=sc[:, 256:512],
                            lhsT=k8[32 * h1:32 * h1 + 32, :, kslc],
                            rhs=q8[32 * h1:32 * h1 + 32, :, qs],
                            perf_mode=PM.DoubleRow,
                            tile_position=(32 * h1, 0))
                        pt = ptp.tile([128, 512], F32R, name="pt", bufs=6)
                        eng = EXP_ENG[ks]
                        if eng == "act":
                            nc.scalar.activation(
                                out=pt, in_=sc, func=AF.Exp, scale=float(SCL))
                        else:
                            e = nc.vector if eng == "dve" else nc.gpsimd
                            e.tensor_scalar(
                                out=pt.bitcast(I32), in0=sc,
                                scalar1=float(EXPA * SCL), scalar2=float(EXPB),
                                op0=OP.mult, op1=OP.add)
                        pts[ks] = pt
                        if ks >= lag:
                            ctx_mms(pts.pop(ks - lag), ks - lag)
                    for ks in range(ST - lag, ST):
                        ctx_mms(pts.pop(ks), ks)

                    # stage ctx -> SBUF; odd head shifts to partitions 64..
                    ctxS = stg.tile([128, 512], F32, name="ctxS", bufs=2)
                    bsh = stg.tile([128, 256], F32, name="bsh", bufs=2)
                    nc.scalar.activation(
                        out=ctxS[0:HD + 1, 0:256], in_=ctxA[0:HD + 1, :],
                        func=AF.Copy)
                    nc.vector.tensor_copy(
                        out=bsh[0:HD + 1, :], in_=ctxB[0:HD + 1, :])
                    nc.sync.dma_start(
                        out=ctxS[64:128, 256:512], in_=bsh[0:HD, :])
                    # broadcast denominators across partitions via PE
                    dn = auxps.tile([128, 256], F32, name="dn", tag="aux")
                    nc.tensor.matmul(
                        out=dn[0:64, :], lhsT=ones64[64:65, :],
                        rhs=ctxS[64:65, 0:256].bitcast(F32R),
                        tile_position=(64, 0), start=True, stop=True)
                    nc.tensor.matmul(
                        out=dn[64:128, :], lhsT=ones64[64:65, :],
                        rhs=bsh[64:65, :].bitcast(F32R),
                        tile_position=(64, 64), start=True, stop=True)
                    rc = small.tile([128, 256], F32, name="rc")
                    nc.vector.reciprocal(out=rc, in_=dn)
                    tmp = stg.tile([128, 256], F32, name="tmp", bufs=3)
                    nc.vector.tensor_mul(
                        out=tmp[0:64, :], in0=ctxS[0:64, 0:256], in1=rc[0:64, :])
                    nc.vector.tensor_mul(
                        out=tmp[64:128, :], in0=ctxS[64:128, 256:512],
                        in1=rc[64:128, :])
                    pend[(qn, hp, sub)] = tmp

                                def division(qn, hp, sub):
                    """Deferred: x += normalized ctx (needs residual in x)."""
                    qs = slice(qn * 512 + sub * 256, qn * 512 + sub * 256 + 256)
                    tmp = pend.pop((qn, hp, sub))
                    nc.vector.tensor_add(
                        out=x[:, hp, qs], in0=x[:, hp, qs], in1=tmp)

                                def stats(qn, sub):
                    qs = slice(qn * 512 + sub * 256, qn * 512 + sub * 256 + 256)
                    nc.vector.tensor_scalar_max(
                        out=x[:, :, qs], in0=x[:, :, qs], scalar1=0.0)
                    x2 = x2p.tile([128, MT, 256], F32R, name="x2")
                    nc.gpsimd.tensor_tensor(
                        out=x2.bitcast(F32), in0=x[:, :, qs], in1=x[:, :, qs],
                        op=OP.mult)
                    sum_ps = auxps.tile([128, 256], F32, name="sum_ps", tag="aux")
                    sq_ps = auxps.tile([128, 256], F32, name="sq_ps", tag="aux")
                    for m in range(MT):
                        nc.tensor.matmul(
                            out=sum_ps[0:1, :], lhsT=onesr,
                            rhs=x[:, m, qs].bitcast(F32R),
                            start=(m == 0), stop=(m == MT - 1))
                    for m in range(MT):
                        nc.tensor.matmul(
                            out=sq_ps[0:1, :], lhsT=onesr, rhs=x2[:, m, :],
                            start=(m == 0), stop=(m == MT - 1))
                    c, half = divmod(qn, 2)
                    off = half * 512 + sub * 256
                    ssb = small.tile([1, 2, 256], F32, name="ssb")
                    nc.scalar.activation(
                        out=ssb[:, 0, :], in_=sum_ps[0:1, :], func=AF.Copy)
                    nc.vector.tensor_copy(out=ssb[:, 1, :], in_=sq_ps[0:1, :])
                    nc.sync.dma_start(
                        out=bass.AP(tensor=cc_in[c].tensor,
                                    offset=cc_in[c].offset + off,
                                    ap=[[1024, 2], [1, 256]]),
                        in_=ssb[0:1, :, :])

                                def collective(c):
                    nc.gpsimd.collective_compute(
                        "AllReduce", mybir.AluOpType.add,
                        replica_groups=[[0, 1, 2, 3], [4, 5, 6, 7]],
                        ins=[cc_in[c][:].opt()],
                        outs=[cc_out[c][:].opt()],
                    )

                def row_math(c, de):
                    """rstd / -mu*rstd for the 1024 tokens of half c."""
                    rsb = rows.tile([64, 2, 16], F32, name=f"rsb{c}")
                    de.dma_start(
                        out=rsb,
                        in_=cc_out[c].rearrange(
                            "r (v p f) -> p (r v) f", v=2, f=16))
                    mean = rows.tile([64, 16], F32, name=f"mean{c}")
                    nc.vector.tensor_scalar_mul(
                        out=mean, in0=rsb[:, 0, :], scalar1=1.0 / D)
                    m2 = rows.tile([64, 16], F32, name=f"m2{c}")
                    nc.vector.tensor_tensor(
                        out=m2, in0=mean, in1=mean, op=OP.mult)
                    var = rows.tile([64, 16], F32, name=f"var{c}")
                    nc.vector.tensor_scalar(
                        out=var, in0=rsb[:, 1, :], scalar1=1.0 / D,
                        scalar2=None, op0=OP.mult, op1=OP.bypass)
                    nc.vector.tensor_tensor(
                        out=var, in0=var, in1=m2, op=OP.subtract)
                    nc.vector.tensor_scalar_add(out=var, in0=var, scalar1=EPS)
                    y = rows.tile([64, 16], F32, name=f"y{c}")
                    nc.vector.tensor_scalar(
                        out=y.bitcast(I32), in0=var.bitcast(I32),
                        scalar1=1, scalar2=-1,
                        op0=OP.arith_shift_right, op1=OP.bitwise_xor)
                    nc.vector.tensor_scalar(
                        out=y.bitcast(I32), in0=y.bitcast(I32),
                        scalar1=0x5F375A86 + 1, scalar2=None,
                        op0=OP.add, op1=OP.bypass)
                    t1 = rows.tile([64, 16], F32, name=f"t1{c}")
                    for _ in range(2):
                        nc.vector.tensor_tensor(
                            out=t1, in0=y, in1=y, op=OP.mult)
                        nc.vector.tensor_tensor(
                            out=t1, in0=t1, in1=var, op=OP.mult)
                        nc.vector.tensor_scalar(
                            out=t1, in0=t1, scalar1=-0.5, scalar2=1.5,
                            op0=OP.mult, op1=OP.add)
                        nc.vector.tensor_tensor(
                            out=y, in0=y, in1=t1, op=OP.mult)
                    nB = rows.tile([64, 2, 16], F32, name=f"nB{c}")
                    nc.vector.tensor_copy(out=nB[:, 0, :], in_=y)
                    nc.vector.tensor_tensor(
                        out=nB[:, 1, :], in0=mean, in1=y, op=OP.mult)
                    nc.vector.tensor_scalar_mul(
                        out=nB[:, 1, :], in0=nB[:, 1, :], scalar1=-1.0)
                    de.dma_start(
                        out=bass.AP(tensor=srow[c].tensor,
                                    offset=srow[c].offset,
                                    ap=[[16, 64], [1024, 2], [1, 16]]),
                        in_=nB)

                                def apply_half(c, de):
                    """LN apply + output for tokens [1024c, 1024c+1024)."""
                    Ab = abp.tile([128, 2048], F32, name="Ab")
                    de.dma_start(
                        out=Ab, in_=bass.AP(
                            tensor=srow[c].tensor,
                            offset=srow[c].offset,
                            ap=[[0, 128], [1, 2048]]))
                    out_t = out_d.rearrange("(t p) s -> p t s", p=128)
                    for half in range(2):
                        hs_ = slice(c * 1024 + half * 512,
                                    c * 1024 + half * 512 + 512)
                        asl = slice(half * 512, half * 512 + 512)
                        bsl = slice(1024 + half * 512, 1536 + half * 512)
                        o = [ostg.tile([128, 512], F32, name="o", bufs=4)
                             for _ in range(MT)]
                        for m in range(MT):
                            e = nc.vector if m == 0 else nc.gpsimd
                            e.tensor_mul(out=o[m], in0=x[:, m, hs_],
                                         in1=Ab[:, asl])
                        for m in range(MT):
                            e = nc.vector if m == 0 else nc.gpsimd
                            e.tensor_add(out=o[m], in0=o[m], in1=Ab[:, bsl])
                        for m in range(MT):
                            nc.scalar.activation(
                                out=o[m], in_=o[m], func=AF.Identity,
                                scale=gm_s[:, m:m + 1], bias=bt_s[:, m:m + 1])
                            de.dma_start(out=out_t[:, m, hs_], in_=o[m])

                                # ---------------- main schedule ----------------
                def qn_round(qn, FA, FB, FC, FD, pri):
                    block(qn, 0, 0, FA)
                    block(qn, 0, 1, FB)
                    block(qn, 1, 0, FC)
                    block(qn, 1, 1, FD)
                    with tc.high_priority() if pri else _null():
                        division(qn, 0, 0)
                        division(qn, 1, 0)
                        stats(qn, 0)
                        division(qn, 0, 1)
                        division(qn, 1, 1)
                        stats(qn, 1)

                from contextlib import nullcontext as _null
                block(0, 0, 0, F000)
                block(0, 0, 1, F001)
                block(0, 1, 0, F010)
                block(0, 1, 1, F011)
                block(1, 0, 0, F100)
                block(1, 0, 1, F101)
                with tc.high_priority():
                    division(0, 0, 0)
                    division(0, 1, 0)
                    stats(0, 0)
                    division(0, 0, 1)
                    division(0, 1, 1)
                    stats(0, 1)
                block(1, 1, 0, F110)
                block(1, 1, 1, F111)
                with tc.high_priority():
                    division(1, 0, 0)
                    division(1, 1, 0)
                    stats(1, 0)
                    division(1, 0, 1)
                    division(1, 1, 1)
                    stats(1, 1)
                    collective(0)
                qn_round(2, F200, F201, [], [], False)
                qn_round(3, [], [], [], [], True)
                with tc.high_priority():
                    collective(1)
                with tc.tile_wait_until(1.0):
                    row_math(0, nc.sync)
                    apply_half(0, nc.sync)
                with tc.tile_wait_until(2.0):
                    row_math(1, nc.scalar)
                    apply_half(1, nc.scalar)

    if split_waits:
        _split_waits(nc)
    return nc


_NC = None
LAST_RESULT = None


def _get_nc():
    global _NC
    if _NC is None:
        _NC = build_bass()
    return _NC


def _dr_pack(a):
    """[1024, C] f32 -> DoubleRow interleave [512, 2C] fp8e4.

    Row kt*128+p, col j*C+c holds a[kt*256 + 128*j + p, c]; the (p, j)
    pairing matches between lhsT and rhs since both use this packing.
    """
    import ml_dtypes
    C = a.shape[1]
    a8 = a.astype(ml_dtypes.float8_e4m3fn)
    return np.ascontiguousarray(
        a8.reshape(4, 2, 128, C).transpose(0, 2, 1, 3).reshape(512, 2 * C))


def kernel(hidden_states, Wq, bq, Wk, bk, Wv, bv, Wp, gamma, beta):
    hs = np.asarray(hidden_states, dtype=np.float32)
    Wq = np.asarray(Wq, np.float32)
    Wk = np.asarray(Wk, np.float32)
    Wv = np.asarray(Wv, np.float32)
    Wp = np.asarray(Wp, np.float32)
    bq = np.asarray(bq, np.float32)
    bk = np.asarray(bk, np.float32)
    bv = np.asarray(bv, np.float32)
    gamma = np.asarray(gamma, np.float32)
    beta = np.asarray(beta, np.float32)

    nc = _get_nc()
    in_maps = []
    for c in range(NCORES):
        b, g = divmod(c, GROUPS)
        sl = slice(g * DC, (g + 1) * DC)
        hsT = hs[b].T  # [1024, 2048]
        in_maps.append({
            "hs8": _dr_pack(hsT),
            "wq8": _dr_pack(Wq[sl].T * WSCALE),
            "wk8": _dr_pack(Wk[sl].T * WSCALE),
            "wv8": _dr_pack(Wv[sl].T * WSCALE),
            "hsT16": np.ascontiguousarray(hsT.astype(np.float16)),
            "wp16": np.ascontiguousarray(Wp[sl].T.astype(np.float16)),
            "bq32": np.ascontiguousarray(bq[sl] * WSCALE),
            "bk32": np.ascontiguousarray(bk[sl] * WSCALE),
            "bv": np.ascontiguousarray(bv[sl]),
            "gamma": np.ascontiguousarray(gamma[sl]),
            "beta": np.ascontiguousarray(beta[sl]),
        })
    res = run_bass_kernel_spmd(nc, in_maps, core_ids=list(range(NCORES)))
    global LAST_RESULT
    LAST_RESULT = res
    out = np.empty((B, S, D), np.float32)
    for c, r in enumerate(res.results):
        b, g = divmod(c, GROUPS)
        out[b, :, g * DC:(g + 1) * DC] = r["outT"].T
    return out
